# revision 15
# baseline (speedup 1.0000x reference)
# Pointer-generator network on 8 trn2 cores.
# Sharding: data-parallel over batch (B=8 -> one batch element per core);
# every per-step op in the reference is batch-independent so no cross-core
# communication is needed. Each core runs embedding gather + BiLSTM encoder +
# attention decoder + vocab softmax + copy-scatter for its b and writes
# out[:, b, :].
#
# Layout notes:
# - recurrences are feature-major: states live as columns [128, j] with the
#   hidden dim on partitions; stored states are 2x-scaled (H2=2h, C2=2c) so
#   sigmoid(x) = 0.5*tanh(x/2)+0.5 folds into cheap scalar_tensor_tensor ops
#   (only the tanh/exp ACT table set is ever used -> no table reloads).
#   The 0.5 factors are folded into pre-scaled weights.
# - gate columns are permuted (i,f,o,g) so one tanh covers all sigmoid gates.
# - attention slab: tanh(P + d) computed as tanh(0.5*P2 + 0.25*d_raw) with the
#   per-step dec_feat entering through the ACT per-partition bias operand.
# - copy distribution: masked dma_scatter_add passes into a v-major DRAM
#   block (bf16), read back transposed via the DMA xbar.
import numpy as np

B, TE, TD, E, H, V, A, MAXOOV = 8, 400, 100, 128, 256, 50000, 612, 50
VEXT = V + MAXOOV
GH = 4 * H
NJ = GH // 128          # 8 gate chunks
NA = (A + 127) // 128   # 5 a-chunks (last partial: 100)
CROWS = 50176           # copy-block rows (>= VEXT, mult of 16)
VCH = 512
NV_FULL = (V + VCH - 1) // VCH  # 98 chunks, last partial 336
PB_CH = 2048
# gate permutation pieces (dst_slice, src_slice) on the 4H axis:
# torch order (i,f,g,o) -> layout (i,f,o,g)
GATE_PIECES = [(slice(0, 512), slice(0, 512)),
               (slice(512, 768), slice(768, 1024)),
               (slice(768, 1024), slice(512, 768))]

_CACHE = {}


def _wrap16(idx, n):
    a = np.asarray(idx, np.int16).reshape(n // 16, 16).T  # [16, n//16]
    return np.ascontiguousarray(np.tile(a, (8, 1)).astype(np.int16))


def _build():
    import concourse.bass as bass
    import concourse.mybir as mybir
    from concourse import bacc
    from concourse.tile import TileContext
    from concourse.masks import make_identity

    f32 = mybir.dt.float32
    bf16 = mybir.dt.bfloat16
    i16 = mybir.dt.int16
    AF = mybir.ActivationFunctionType
    OP = mybir.AluOpType
    ds = bass.ds

    nc = bacc.Bacc('TRN2', target_bir_lowering=False, debug=False)

    def din(name, shape, dt=f32):
        return nc.dram_tensor(name, shape, dt, kind='ExternalInput')

    embedding = din('embedding', [V, E])
    Wih = {d: din(f'Wih_{d}', [E, GH]) for d in 'fbd'}
    Whh = {d: din(f'Whh_{d}', [H, GH]) for d in 'fbd'}
    bias_g = {d: din(f'b_{d}', [GH]) for d in 'fbd'}
    Wred = din('Wred', [2 * H, H])
    bred = din('bred', [H])
    Wenc = din('Wenc_feat', [2 * H, A])
    benc = din('benc_feat', [A])
    v_in = din('v', [A])
    Wx = din('Wx', [E + 2 * H, E])
    bx = din('bx', [E])
    Wdf = din('Wdf', [TD, 2 * H, A])
    bdf = din('bdf', [TD, A])
    Wpg = din('Wpg', [E + 4 * H, 1])
    bpg = din('bpg', [1])
    Wout = din('Wout', [3 * H, H])
    bout = din('bout', [H])
    Wv = din('Wv', [H, V])
    bv = din('bv', [V])
    enc_ilo = din('enc_ilo', [128, 32], i16)
    enc_ihi = din('enc_ihi', [128, 32], i16)
    enc_msk = din('enc_msk', [128, 4], f32)
    dec_ilo = din('dec_ilo', [128, 8], i16)
    dec_ihi = din('dec_ihi', [128, 8], i16)
    dec_msk = din('dec_msk', [128, 1], f32)
    attn_mask = din('attn_mask', [1, TE], f32)
    sc_idx = [din(f'sc_idx{p}', [128, 32], i16) for p in range(6)]
    sc_msk = [din(f'sc_msk{p}', [128, 4], f32) for p in range(6)]

    out_b = nc.dram_tensor('out_b', [TD, VEXT], f32, kind='ExternalOutput')
    cblk = nc.dram_tensor('cblk', [CROWS, 128], bf16)

    from contextlib import ExitStack
    with TileContext(nc) as tc, ExitStack() as stk:
        per = stk.enter_context(tc.tile_pool(name='per', bufs=1))
        ps1 = stk.enter_context(tc.tile_pool(name='ps1', bufs=1, space='PSUM'))

        ident = per.tile([128, 128], f32)
        make_identity(nc, ident[:, :])

        # ---- persistent smalls ------------------------------------------
        whh_d_sb = per.tile([128, 2, NJ, 128], bf16)
        wih_d_bf = per.tile([128, NJ, 128], bf16)
        bT_d = per.tile([128, NJ], f32)
        wxb_sb = per.tile([128, 4, 128], bf16)
        bxT = per.tile([128, 1], f32)
        wpg_sb = per.tile([128, 9], f32)
        bpg_sb = per.tile([1, 1], f32)
        wout_sb = per.tile([128, 6, 2, 128], f32)
        boutT = per.tile([128, 2], f32)
        msk_sb = per.tile([1, TE], f32)
        v_sb = per.tile([128, NA], f32)
        p2 = per.tile([128, NA, TE], f32)
        encout_pm = per.tile([128, 4, 512], f32)
        stateT = per.tile([128, 9, TD], f32)
        wTn = per.tile([128, 4, TD], f32)
        outsT = per.tile([128, 2, TD], f32)
        x_embT = per.tile([128, TD], f32)
        a_dec = per.tile([128, NJ, TD], f32)
        bdfT_all = per.tile([128, NA, TD], f32)
        h2_d = per.tile([128, 2], f32)
        c2_d = per.tile([128, 2], f32)
        pgT = per.tile([TD, 1], f32)

        def load_gate_perm_hh(dst_ap4, src2d):
            # dst [128, 2, NJ, 128]: dst[p,k,m,c] = src[128k+p, perm(128m+c)]
            for (dsl, ssl) in GATE_PIECES:
                nc.gpsimd.dma_start(
                    out=dst_ap4.rearrange('p k m c -> p k (m c)')[:, :, dsl],
                    in_=src2d[:, ssl].rearrange('(k p) c -> p k c', p=128))

        def load_gate_perm_ih(dst_ap3, src2d):
            # dst [128, NJ, 128]: dst[p,m,c] = src[p, perm(128m+c)]
            for (dsl, ssl) in GATE_PIECES:
                nc.gpsimd.dma_start(
                    out=dst_ap3.rearrange('p m c -> p (m c)')[:, dsl],
                    in_=src2d[:, ssl])

        # =========== encoder-scoped pools ================================
        with tc.tile_pool(name='encp', bufs=1) as encp, \
                tc.tile_pool(name='enct', bufs=2) as enct, \
                tc.tile_pool(name='psenc', bufs=1, space='PSUM') as psenc:
            # -- weight prep
            whh_sb = {}
            wih_sb = {}
            bT = {}
            for d in 'fb':
                w = encp.tile([128, 2, NJ, 128], bf16, name=f'whh_{d}')
                load_gate_perm_hh(w[:, :, :, :], Whh[d])
                nc.vector.tensor_scalar_mul(w[:, :, :, :], w[:, :, :, :], 0.5)
                whh_sb[d] = w
                wi = encp.tile([128, NJ, 128], f32, name=f'wih_{d}')
                load_gate_perm_ih(wi[:, :, :], Wih[d])
                wih_sb[d] = wi
                bt = encp.tile([128, NJ], f32, name=f'bT_{d}')
                for (dsl, ssl) in GATE_PIECES:
                    nc.sync.dma_start(
                        out=bt[:, dsl.start // 128:dsl.stop // 128],
                        in_=bias_g[d][ssl].rearrange('(j p) -> p j', p=128))
                bT[d] = bt
            # decoder weights (persistent)
            load_gate_perm_hh(whh_d_sb[:, :, :, :], Whh['d'])
            nc.vector.tensor_scalar_mul(whh_d_sb[:, :, :, :],
                                        whh_d_sb[:, :, :, :], 0.5)
            wih_d_f32 = encp.tile([128, NJ, 128], f32, name='wih_d32')
            load_gate_perm_ih(wih_d_f32[:, :, :], Wih['d'])
            nc.vector.tensor_copy(wih_d_bf[:, :, :], wih_d_f32[:, :, :])
            for (dsl, ssl) in GATE_PIECES:
                nc.sync.dma_start(
                    out=bT_d[:, dsl.start // 128:dsl.stop // 128],
                    in_=bias_g['d'][ssl].rearrange('(j p) -> p j', p=128))
            wred_sb = encp.tile([128, 4, 2, 128], f32, name='wred')
            nc.sync.dma_start(
                out=wred_sb[:, :, :, :],
                in_=Wred.ap().rearrange('(k p) (m c) -> p k m c', p=128, c=128))
            bredT = encp.tile([128, 2], f32, name='bredT')
            nc.sync.dma_start(
                out=bredT[:, :],
                in_=bred.ap().rearrange('(j p) -> p j', p=128))
            wenc_sb = encp.tile([128, 4, A], f32, name='wenc')
            nc.sync.dma_start(
                out=wenc_sb[:, :, :],
                in_=Wenc.ap().rearrange('(k p) c -> p k c', p=128))
            bencT = encp.tile([128, NA], f32, name='bencT')
            nc.vector.memset(bencT[:, :], 0.0)
            nc.sync.dma_start(
                out=bencT[:, 0:4],
                in_=benc[0:512].rearrange('(j p) -> p j', p=128))
            nc.sync.dma_start(out=bencT[0:A - 512, 4, None],
                              in_=benc[512:A, None])
            nc.vector.memset(v_sb[:, :], 0.0)
            nc.sync.dma_start(
                out=v_sb[:, 0:4],
                in_=v_in[0:512].rearrange('(j p) -> p j', p=128))
            nc.sync.dma_start(out=v_sb[0:A - 512, 4, None],
                              in_=v_in[512:A, None])
            wxt_sb = encp.tile([128, 128], f32, name='wxt')
            nc.sync.dma_start(out=wxt_sb[:, :], in_=Wx[0:E, :])
            nc.gpsimd.dma_start(
                out=wxb_sb[:, :, :],
                in_=Wx[E:, :].rearrange('(k p) c -> p k c', p=128))
            nc.vector.tensor_scalar_mul(wxb_sb[:, :, :], wxb_sb[:, :, :], 0.5)
            nc.sync.dma_start(out=bxT[:, :], in_=bx[:, None])
            nc.sync.dma_start(
                out=wout_sb[:, :, :, :],
                in_=Wout.ap().rearrange('(k p) (m c) -> p k m c',
                                        p=128, c=128))
            nc.vector.tensor_scalar_mul(wout_sb[:, :, :, :],
                                        wout_sb[:, :, :, :], 0.5)
            nc.sync.dma_start(
                out=boutT[:, :],
                in_=bout.ap().rearrange('(j p) -> p j', p=128))
            nc.sync.dma_start(
                out=wpg_sb[:, :],
                in_=Wpg.ap().rearrange('(k p) one -> p (k one)', p=128))
            nc.vector.tensor_scalar_mul(wpg_sb[:, 0:8], wpg_sb[:, 0:8], 0.5)
            nc.sync.dma_start(out=bpg_sb[:, :], in_=bpg[None, :])
            nc.vector.tensor_scalar_mul(bpg_sb[:, :], bpg_sb[:, :], 0.5)
            nc.sync.dma_start(out=msk_sb[:, :], in_=attn_mask[:, :])
            nc.vector.memset(bdfT_all[:, :, :], 0.0)
            bdf_pm = enct.tile([TD, A], f32, name='bdfpm')
            nc.sync.dma_start(out=bdf_pm[:, :], in_=bdf[:, :])
            for j in range(NA):
                jw = min(128, A - 128 * j)
                pbt = psenc.tile([128, TD], f32, name='pa')
                nc.tensor.transpose(pbt[0:jw, :],
                                    bdf_pm[:, 128 * j:128 * j + jw],
                                    ident[0:TD, 0:TD])
                nc.scalar.copy(bdfT_all[0:jw, j, :], pbt[0:jw, :])

            # -- embedding gathers
            def gather_tokens(ilo, ihi, mskd, nidx, npq):
                glo = enct.tile([128, npq, 128], f32, name='glo')
                ghi = enct.tile([128, npq, 128], f32, name='ghi')
                isb_lo = enct.tile([128, nidx // 16], i16, name='gilo')
                isb_hi = enct.tile([128, nidx // 16], i16, name='gihi')
                msb = enct.tile([128, npq], f32, name='gmsk')
                nc.sync.dma_start(out=isb_lo[:, :], in_=ilo[:, 0:nidx // 16])
                nc.sync.dma_start(out=isb_hi[:, :], in_=ihi[:, 0:nidx // 16])
                nc.sync.dma_start(out=msb[:, :], in_=mskd[:, 0:npq])
                nc.gpsimd.dma_gather(glo[:, :, :], embedding[:, :],
                                     isb_lo[:, :], nidx, nidx, E)
                nc.gpsimd.dma_gather(ghi[:, :, :], embedding[32768:, :],
                                     isb_hi[:, :], nidx, nidx, E)
                dif = enct.tile([128, npq, 128], f32, name='gdif')
                nc.vector.tensor_tensor(dif[:, :, :], ghi[:, :, :],
                                        glo[:, :, :], OP.subtract)
                nc.vector.tensor_tensor(
                    dif[:, :, :], dif[:, :, :],
                    msb[:, :, None].broadcast_to([128, npq, 128]), OP.mult)
                nc.vector.tensor_tensor(glo[:, :, :], glo[:, :, :],
                                        dif[:, :, :], OP.add)
                return glo

            xe = gather_tokens(enc_ilo, enc_ihi, enc_msk, 512, 4)
            xd = gather_tokens(dec_ilo, dec_ihi, dec_msk, 128, 1)
            xeT = encp.tile([128, 4, 128], f32, name='xeT')
            for q in range(4):
                pt = psenc.tile([128, 128], f32, name='ptr')
                nc.tensor.transpose(pt[:, :], xe[:, q, :], ident[:, :])
                nc.scalar.copy(xeT[:, q, :], pt[:, :])
            xdT = encp.tile([128, 128], f32, name='xdT')
            ptd = psenc.tile([128, 128], f32, name='ptr')
            nc.tensor.transpose(ptd[:, :], xd[:, 0, :], ident[:, :])
            nc.scalar.copy(xdT[:, :], ptd[:, :])

            # -- a = x@Wih + b (feature-major; bwd stored position-reversed)
            a_enc = {}
            for d in 'fb':
                a = encp.tile([128, NJ, TE], f32, name=f'a_{d}')
                for m in range(NJ):
                    pa = psenc.tile([128, TE], f32, name='pa')
                    nc.tensor.matmul(
                        pa[:, :], wih_sb[d][:, m, :],
                        xeT[:, :, :].rearrange('p q c -> p (q c)')[:, 0:TE])
                    if d == 'f':
                        nc.vector.tensor_scalar_add(a[:, m, :], pa[:, :],
                                                    bT[d][:, m, None])
                    else:
                        nc.vector.tensor_scalar_add(a[:, m, ::-1], pa[:, :],
                                                    bT[d][:, m, None])
                a_enc[d] = a
            pxe = psenc.tile([128, TD], f32, name='pa')
            nc.tensor.matmul(pxe[:, :], wxt_sb[:, :], xdT[:, 0:TD])
            nc.vector.tensor_scalar_add(x_embT[:, :], pxe[:, :], bxT[:, :])
            for m in range(NJ):
                pa = psenc.tile([128, TD], f32, name='pa')
                nc.tensor.matmul(pa[:, :], wih_d_f32[:, m, :], x_embT[:, :])
                nc.vector.tensor_scalar_add(a_dec[:, m, :], pa[:, :],
                                            bT_d[:, m, None])

            # -- encoder BiLSTM --------------------------------------------
            encoutT = encp.tile([128, 4, TE], f32, name='encoutT')
            encoutT_bwd = encp.tile([128, 2, TE], f32, name='encoutT_b')
            st = {}
            for d in 'fb':
                for nm in ('h2', 'c2'):
                    t_ = encp.tile([128, 2], f32, name=f'{nm}_{d}')
                    nc.vector.memset(t_[:, :], 0.0)
                    st[f'{nm}_{d}'] = t_
                tb = encp.tile([128, 2], bf16, name=f'h2b_{d}')
                nc.vector.memset(tb[:, :], 0.0)
                st[f'h2b_{d}'] = tb

            def lstm_step(pref, whh_t, a_t, h2, c2, h2b, hout, pool, pspool):
                pg = pspool.tile([128, NJ], f32, name=f'pg{pref}')
                for m in range(NJ):
                    for k in range(2):
                        nc.tensor.matmul(pg[:, m, None],
                                         whh_t[:, k, m, :],
                                         h2b[:, k, None], start=(k == 0),
                                         stop=(k == 1))
                g = pool.tile([128, NJ], f32, name=f'g{pref}')
                nc.vector.tensor_tensor(g[:, :], pg[:, :], a_t, OP.add)
                t_ifo = pool.tile([128, 6], f32, name=f'ti{pref}')
                nc.scalar.activation(t_ifo[:, :], g[:, 0:6], AF.Tanh,
                                     scale=0.5)
                t_g = pool.tile([128, 2], f32, name=f'tg{pref}')
                nc.scalar.activation(t_g[:, :], g[:, 6:8], AF.Tanh)
                u1 = pool.tile([128, 2], f32, name=f'u1{pref}')
                u2 = pool.tile([128, 2], f32, name=f'u2{pref}')
                nc.vector.scalar_tensor_tensor(u1[:, :], t_ifo[:, 2:4], 1.0,
                                               c2[:, :], OP.add, OP.mult)
                nc.vector.scalar_tensor_tensor(u2[:, :], t_ifo[:, 0:2], 1.0,
                                               t_g[:, :], OP.add, OP.mult)
                nc.vector.scalar_tensor_tensor(c2[:, :], u1[:, :], 0.5,
                                               u2[:, :], OP.mult, OP.add)
                th = pool.tile([128, 2], f32, name=f'th{pref}')
                nc.scalar.activation(th[:, :], c2[:, :], AF.Tanh, scale=0.5)
                nc.vector.scalar_tensor_tensor(hout, t_ifo[:, 4:6], 1.0,
                                               th[:, :], OP.add, OP.mult)
                nc.vector.tensor_copy(h2[:, :], hout)
                nc.vector.tensor_copy(h2b[:, :], hout)

            ENC_UNROLL = 4
            with tc.For_i(0, TE, ENC_UNROLL) as it0:
                for u_i in range(ENC_UNROLL):
                    iv = it0 + u_i
                    lstm_step(
                        'f', whh_sb['f'],
                        a_enc['f'][:, :, ds(iv, 1)].rearrange(
                            'p j one -> p (j one)'),
                        st['h2_f'], st['c2_f'], st['h2b_f'],
                        encoutT[:, 0:2, ds(iv, 1)].rearrange(
                            'p j one -> p (j one)'), enct, psenc)
                    lstm_step(
                        'b', whh_sb['b'],
                        a_enc['b'][:, :, ds(iv, 1)].rearrange(
                            'p j one -> p (j one)'),
                        st['h2_b'], st['c2_b'], st['h2b_b'],
                        encoutT_bwd[:, :, ds(iv, 1)].rearrange(
                            'p j one -> p (j one)'), enct, psenc)
            nc.vector.tensor_copy(encoutT[:, 2:4, :], encoutT_bwd[:, :, ::-1])

            # -- reducer: 2*h0 = relu([2hf|2hb]@Wred + 2*bred)
            bred2 = enct.tile([128, 2], f32, name='bred2')
            nc.vector.tensor_scalar_mul(bred2[:, :], bredT[:, :], 2.0)
            for (dst, sf, sb) in ((h2_d, st['h2_f'], st['h2_b']),
                                  (c2_d, st['c2_f'], st['c2_b'])):
                pr = psenc.tile([128, 2], f32, name='pp2')
                for m in range(2):
                    for k in range(4):
                        src = sf if k < 2 else sb
                        nc.tensor.matmul(pr[:, m, None],
                                         wred_sb[:, k, m, :],
                                         src[:, k % 2, None], start=(k == 0),
                                         stop=(k == 3))
                for m in range(2):
                    nc.scalar.activation(dst[:, m, None], pr[:, m, None],
                                         AF.Relu, bias=bred2[:, m, None])

            # -- P2 slab + pos-major enc_out
            benc2 = enct.tile([128, NA], f32, name='benc2')
            nc.vector.tensor_scalar_mul(benc2[:, :], bencT[:, :], 2.0)
            nc.vector.memset(p2[:, :, :], 0.0)
            for m in range(NA):
                mw = min(128, A - 128 * m)
                pp = psenc.tile([128, TE], f32, name='pp2')
                for k in range(4):
                    nc.tensor.matmul(pp[0:mw, :],
                                     wenc_sb[:, k, 128 * m:128 * m + mw],
                                     encoutT[:, k, :], start=(k == 0),
                                     stop=(k == 3))
                nc.vector.tensor_scalar_add(p2[0:mw, m, :], pp[0:mw, :],
                                            benc2[0:mw, m, None])
            nc.vector.memset(encout_pm[:, :, :], 0.0)
            for q in range(4):
                pw_ = min(128, TE - 128 * q)
                for dj in range(4):
                    pt = psenc.tile([128, 128], f32, name='ptr')
                    nc.tensor.transpose(
                        pt[0:pw_, :],
                        encoutT[:, dj, 128 * q:128 * q + pw_], ident[:, :])
                    nc.scalar.copy(
                        encout_pm[0:pw_, q, 128 * dj:128 * (dj + 1)],
                        pt[0:pw_, :])

        # =========== decoder ==============================================
        with tc.tile_pool(name='decp', bufs=1) as decp, \
                tc.tile_pool(name='dect', bufs=2) as dect, \
                tc.tile_pool(name='wdfp', bufs=2) as wdfp, \
                tc.tile_pool(name='psdec', bufs=1, space='PSUM') as psdec:
            ctx2T = decp.tile([128, 4], f32)
            ctx2Tb = decp.tile([128, 4], bf16)
            nc.vector.memset(ctx2T[:, :], 0.0)
            nc.vector.memset(ctx2Tb[:, :], 0.0)
            h2b_d = decp.tile([128, 2], bf16)
            hc2b = decp.tile([128, 4], bf16)
            nc.vector.tensor_copy(h2b_d[:, :], h2_d[:, :])
            nc.vector.tensor_copy(hc2b[:, 0:2], h2_d[:, :])
            nc.vector.tensor_copy(hc2b[:, 2:4], c2_d[:, :])
            slab = decp.tile([128, NA, TE], f32)
            w = decp.tile([1, 512], f32)
            nc.vector.memset(w[:, :], 0.0)

            with tc.For_i(0, TD, 1) as t:
                px = psdec.tile([128, 1], f32, name='px')
                for k in range(4):
                    nc.tensor.matmul(px[:, :], wxb_sb[:, k, :],
                                     ctx2Tb[:, k, None], start=(k == 0),
                                     stop=(k == 3))
                ctxpT = dect.tile([128, 1], f32, name='ctxp')
                nc.vector.tensor_copy(ctxpT[:, :], px[:, :])
                ctxpTb = dect.tile([128, 1], bf16, name='ctxpb')
                nc.vector.tensor_copy(ctxpTb[:, :], ctxpT[:, :])
                nc.vector.tensor_tensor(stateT[:, 8, ds(t, 1)], ctxpT[:, :],
                                        x_embT[:, ds(t, 1)], OP.add)
                pg = psdec.tile([128, NJ], f32, name='pgd')
                for m in range(NJ):
                    nc.tensor.matmul(pg[:, m, None], wih_d_bf[:, m, :],
                                     ctxpTb[:, :], start=True, stop=False)
                    for k in range(2):
                        nc.tensor.matmul(pg[:, m, None],
                                         whh_d_sb[:, k, m, :],
                                         h2b_d[:, k, None], start=False,
                                         stop=(k == 1))
                g = dect.tile([128, NJ], f32, name='gd')
                nc.vector.tensor_tensor(
                    g[:, :], pg[:, :],
                    a_dec[:, :, ds(t, 1)].rearrange('p j one -> p (j one)'),
                    OP.add)
                t_ifo = dect.tile([128, 6], f32, name='tifod')
                nc.scalar.activation(t_ifo[:, :], g[:, 0:6], AF.Tanh,
                                     scale=0.5)
                t_g = dect.tile([128, 2], f32, name='tgd')
                nc.scalar.activation(t_g[:, :], g[:, 6:8], AF.Tanh)
                u1 = dect.tile([128, 2], f32, name='u1d')
                u2 = dect.tile([128, 2], f32, name='u2d')
                nc.vector.scalar_tensor_tensor(u1[:, :], t_ifo[:, 2:4], 1.0,
                                               c2_d[:, :], OP.add, OP.mult)
                nc.vector.scalar_tensor_tensor(u2[:, :], t_ifo[:, 0:2], 1.0,
                                               t_g[:, :], OP.add, OP.mult)
                nc.vector.scalar_tensor_tensor(c2_d[:, :], u1[:, :], 0.5,
                                               u2[:, :], OP.mult, OP.add)
                th = dect.tile([128, 2], f32, name='thd')
                nc.scalar.activation(th[:, :], c2_d[:, :], AF.Tanh, scale=0.5)
                nc.vector.scalar_tensor_tensor(h2_d[:, :], t_ifo[:, 4:6], 1.0,
                                               th[:, :], OP.add, OP.mult)
                nc.vector.tensor_copy(stateT[:, 4:6, ds(t, 1)], h2_d[:, :])
                nc.vector.tensor_copy(stateT[:, 6:8, ds(t, 1)], c2_d[:, :])
                nc.vector.tensor_copy(h2b_d[:, :], h2_d[:, :])
                nc.vector.tensor_copy(hc2b[:, 0:2], h2_d[:, :])
                nc.vector.tensor_copy(hc2b[:, 2:4], c2_d[:, :])
                wdf_t = wdfp.tile([128, 4, A], bf16, name='wdft')
                nc.gpsimd.dma_start(
                    out=wdf_t[:, :, :],
                    in_=Wdf[ds(t, 1), :, :].rearrange(
                        'one (k p) a -> p (one k) a', p=128))
                pd = psdec.tile([128, NA], f32, name='pd')
                for m in range(NA):
                    mw = min(128, A - 128 * m)
                    for k in range(4):
                        nc.tensor.matmul(pd[0:mw, m, None],
                                         wdf_t[:, k, 128 * m:128 * m + mw],
                                         hc2b[:, k, None], start=(k == 0),
                                         stop=(k == 3))
                d4 = dect.tile([128, NA], f32, name='d4')
                nc.vector.scalar_tensor_tensor(
                    d4[:, :], pd[:, :], 0.25,
                    bdfT_all[:, :, ds(t, 1)].rearrange(
                        'p j one -> p (j one)'), OP.mult, OP.bypass)
                nc.vector.scalar_tensor_tensor(
                    d4[:, :],
                    bdfT_all[:, :, ds(t, 1)].rearrange('p j one -> p (j one)'),
                    0.5, d4[:, :], OP.mult, OP.add)
                pe = psdec.tile([1, TE], f32, name='pe')
                for m in range(NA):
                    nc.scalar.activation(slab[:, m, :], p2[:, m, :], AF.Tanh,
                                         scale=0.5, bias=d4[:, m, None])
                    nc.tensor.matmul(pe[:, :], v_sb[:, m, None],
                                     slab[:, m, :], start=(m == 0),
                                     stop=(m == NA - 1))
                nc.scalar.activation(w[:, 0:TE], pe[:, :], AF.Exp)
                S = dect.tile([1, 1], f32, name='S')
                nc.vector.scalar_tensor_tensor(w[:, 0:TE], w[:, 0:TE], 1.0,
                                               msk_sb[:, :], OP.mult, OP.mult,
                                               accum_out=S[:, :])
                invS = dect.tile([1, 1], f32, name='invS')
                nc.vector.reciprocal(invS[:, :], S[:, :])
                nc.vector.tensor_scalar_mul(w[:, 0:TE], w[:, 0:TE],
                                            invS[:, :])
                pw = psdec.tile([128, 4], f32, name='pw')
                for q in range(4):
                    nc.tensor.transpose(pw[:, q, None],
                                        w[:, 128 * q:128 * (q + 1)],
                                        ident[0:1, 0:1])
                wn = dect.tile([128, 4], f32, name='wn')
                nc.vector.tensor_copy(wn[:, :], pw[:, :])
                nc.vector.tensor_copy(wTn[:, :, ds(t, 1)], wn[:, :])
                pc = psdec.tile([1, 512], f32, name='pc')
                for q in range(4):
                    nc.tensor.matmul(pc[:, :], wn[:, q, None],
                                     encout_pm[:, q, :], start=(q == 0),
                                     stop=(q == 3))
                csb = dect.tile([1, 512], f32, name='csb')
                nc.vector.tensor_copy(csb[:, :], pc[:, :])
                pct = psdec.tile([128, 4], f32, name='pct')
                for dj in range(4):
                    nc.tensor.transpose(pct[:, dj, None],
                                        csb[:, 128 * dj:128 * (dj + 1)],
                                        ident[0:1, 0:1])
                nc.vector.tensor_copy(ctx2T[:, :], pct[:, :])
                nc.vector.tensor_copy(ctx2Tb[:, :], ctx2T[:, :])
                nc.vector.tensor_copy(stateT[:, 0:4, ds(t, 1)], ctx2T[:, :])

        # =========== outs / pgen / copy-scatter ===========================
        with tc.tile_pool(name='posp', bufs=2) as posp, \
                tc.tile_pool(name='pspos', bufs=1, space='PSUM') as pspos:
            for m in range(2):
                po = pspos.tile([128, TD], f32, name='po')
                for k in range(6):
                    src = stateT[:, 4 + k, :] if k < 2 else stateT[:, k - 2, :]
                    nc.tensor.matmul(po[:, :], wout_sb[:, k, m, :], src,
                                     start=(k == 0), stop=(k == 5))
                nc.vector.tensor_scalar_add(outsT[:, m, :], po[:, :],
                                            boutT[:, m, None])
            pz = pspos.tile([1, TD], f32, name='pz')
            for k in range(9):
                nc.tensor.matmul(pz[:, :], wpg_sb[:, k, None],
                                 stateT[:, k, :], start=(k == 0),
                                 stop=(k == 8))
            tpg = posp.tile([1, TD], f32, name='tpg')
            nc.scalar.activation(tpg[:, :], pz[:, :], AF.Tanh, scale=0.5,
                                 bias=bpg_sb[0:1, 0:1])
            ppgT = pspos.tile([TD, 1], f32, name='ppgT')
            nc.tensor.transpose(ppgT[:, :], tpg[:, :], ident[0:1, 0:1])
            nc.vector.tensor_scalar(pgT[:, :], ppgT[:, :], 0.5, 0.5, OP.mult,
                                    OP.add)
            # (1-pgen) broadcast to all partitions via ones-matmul
            ponesT = posp.tile([1, 128], f32, name='ponesT')
            nc.vector.memset(ponesT[:, :], 1.0)
            omp_ps = pspos.tile([128, TD], f32, name='omp_ps')
            nc.tensor.matmul(omp_ps[:, :], ponesT[:, :], tpg[:, :])
            ompB = posp.tile([128, TD], f32, name='ompB')
            nc.vector.tensor_scalar(ompB[:, :], omp_ps[:, :], -0.5, 0.5,
                                    OP.mult, OP.add)
            wval = posp.tile([128, 4, 128], bf16, name='wval')
            nc.vector.memset(wval[:, :, :], 0.0)
            nc.vector.tensor_tensor(
                wval[:, :, 0:TD], wTn[:, :, :],
                ompB[:, None, :].broadcast_to([128, 4, TD]), OP.mult)
            zt = posp.tile([128, 1024], bf16, name='zt')
            nc.vector.memset(zt[:, :], 0.0)
            for r in range(0, CROWS, 1024):
                nc.sync.dma_start(
                    out=cblk[r:r + 1024, :].rearrange('(q p) c -> p q c',
                                                      p=128),
                    in_=zt[:, :].rearrange('p (q c) -> p q c', c=128))
            for p_i in range(6):
                isb = posp.tile([128, 32], i16, name='scidx')
                msb = posp.tile([128, 4], f32, name='scmsk')
                nc.sync.dma_start(out=isb[:, :], in_=sc_idx[p_i][:, :])
                nc.sync.dma_start(out=msb[:, :], in_=sc_msk[p_i][:, :])
                pv = posp.tile([128, 4, 128], bf16, name='pv')
                nc.vector.tensor_tensor(
                    pv[:, :, :], wval[:, :, :],
                    msb[:, :, None].broadcast_to([128, 4, 128]), OP.mult)
                base = 32768 if p_i % 2 == 1 else 0
                nc.gpsimd.dma_scatter_add(cblk[base:, :], pv[:, :, :],
                                          isb[:, :], 512, 512, 128)

        # =========== vocab dist + final ===================================
        with tc.tile_pool(name='ph5p', bufs=1) as ph5p, \
                tc.tile_pool(name='ph5s', bufs=3) as ph5s, \
                tc.tile_pool(name='ps5', bufs=3, space='PSUM') as ps5:
            u_slab = ph5p.tile([128, V], bf16)
            sums = ph5p.tile([TD, NV_FULL], f32)
            ones1 = ph5p.tile([1, TD], f32)
            nc.vector.memset(ones1[:, :], 1.0)
            for c in range(NV_FULL):
                v0 = c * VCH
                vw = min(VCH, V - v0)
                wvt = ph5s.tile([128, 2, VCH], f32, name='wvt')
                nc.sync.dma_start(
                    out=wvt[:, :, 0:vw],
                    in_=Wv[:, v0:v0 + vw].rearrange('(k p) c -> p k c',
                                                    p=128))
                bvc = ph5s.tile([1, VCH], f32, name='bvc')
                nc.sync.dma_start(out=bvc[:, 0:vw], in_=bv[None, v0:v0 + vw])
                pl = ps5.tile([TD, VCH], f32, name='pl')
                for k in range(2):
                    nc.tensor.matmul(pl[:, 0:vw], outsT[:, k, :],
                                     wvt[:, k, 0:vw], start=(k == 0),
                                     stop=False)
                nc.tensor.matmul(pl[:, 0:vw], ones1[:, :],
                                 bvc[:, 0:vw], start=False, stop=True)
                nc.scalar.activation(u_slab[0:TD, v0:v0 + vw], pl[:, 0:vw],
                                     AF.Exp, accum_out=sums[:, c, None])
            rowsum = ph5p.tile([TD, 1], f32)
            nc.vector.tensor_reduce(rowsum[:, :], sums[:, :],
                                    axis=mybir.AxisListType.X, op=OP.add)
            rscale = ph5p.tile([TD, 1], f32)
            nc.vector.reciprocal(rscale[:, :], rowsum[:, :])
            nc.vector.tensor_tensor(rscale[:, :], rscale[:, :], pgT[:, :],
                                    OP.mult)
            NB = (V + PB_CH - 1) // PB_CH
            for c in range(NB):
                v0 = c * PB_CH
                vw = min(PB_CH, V - v0)
                ct = ph5s.tile([128, PB_CH], bf16, name='ct')
                nc.sync.dma_start_transpose(ct[:, 0:vw], cblk[v0:v0 + vw, :])
                och = ph5s.tile([TD, PB_CH], f32, name='och')
                nc.vector.scalar_tensor_tensor(
                    och[:, 0:vw], u_slab[0:TD, v0:v0 + vw], rscale[:, :],
                    ct[0:TD, 0:vw], OP.mult, OP.add)
                nc.sync.dma_start(out=out_b[:, v0:v0 + vw], in_=och[:, 0:vw])
            ctt = ph5s.tile([128, 64], bf16, name='ctt')
            nc.sync.dma_start_transpose(ctt[:, :], cblk[V:V + 64, :])
            otail = ph5s.tile([TD, MAXOOV], f32, name='otail')
            nc.vector.tensor_copy(otail[:, :], ctt[0:TD, 0:MAXOOV])
            nc.sync.dma_start(out=out_b[:, V:VEXT], in_=otail[:, :])
    nc.finalize()
    return nc


def _prep_inmaps(inputs):
    ins = {k: np.asarray(v) for k, v in inputs.items()}
    wkeys = ('embedding', 'Wih_f', 'Whh_f', 'b_f', 'Wih_b', 'Whh_b', 'b_b',
             'Wred', 'bred', 'Wenc_feat', 'benc_feat', 'v', 'Wx', 'bx',
             'Wih_d', 'Whh_d', 'b_d', 'Wdf', 'bdf', 'Wpg', 'bpg', 'Wout',
             'bout', 'Wv', 'bv')
    shared = {k: np.ascontiguousarray(ins[k], np.float32) for k in wkeys}
    maps = []
    for b in range(B):
        m = dict(shared)
        et = ins['encoder_tokens'][b].astype(np.int64)
        dt_ = ins['decoder_tokens'][b].astype(np.int64)

        def tok_split(toks, n):
            t = np.zeros(n, np.int64)
            t[:len(toks)] = toks
            lo = np.where(t < 32768, t, 0)
            hi = np.where(t >= 32768, t - 32768, 0)
            mq = (t >= 32768).astype(np.float32).reshape(n // 128, 128).T
            return (_wrap16(lo, n), _wrap16(hi, n), np.ascontiguousarray(mq))
        m['enc_ilo'], m['enc_ihi'], m['enc_msk'] = tok_split(et, 512)
        m['dec_ilo'], m['dec_ihi'], m['dec_msk'] = tok_split(dt_, 128)
        m['attn_mask'] = np.ascontiguousarray(
            ins['encoder_mask'][b:b + 1], np.float32)
        ext = ins['encoder_ext_tokens'][b].astype(np.int64)
        occ = {}
        passes = [(np.zeros(512, np.int64), np.zeros(512, np.float32))
                  for _ in range(6)]
        for j, vv in enumerate(ext):
            o = occ.get(vv, 0)
            occ[vv] = o + 1
            assert o < 3, "more than 3 duplicates of one ext token"
            hi = 1 if vv >= 32768 else 0
            pi = 2 * o + hi
            passes[pi][0][j] = vv - 32768 * hi
            passes[pi][1][j] = 1.0
        for p_i in range(6):
            idx, mskv = passes[p_i]
            m[f'sc_idx{p_i}'] = _wrap16(idx, 512)
            m[f'sc_msk{p_i}'] = np.ascontiguousarray(
                mskv.reshape(4, 128).T.astype(np.float32))
        maps.append(m)
    return maps


def kernel(**inputs):
    import os
    from concourse.bass_utils import run_bass_kernel_spmd
    if 'nc' not in _CACHE:
        _CACHE['nc'] = _build()
    in_maps = _prep_inmaps(inputs)
    kw = {}
    if os.environ.get('PGNET_TRACE'):
        kw = dict(trace=True)
    res = run_bass_kernel_spmd(_CACHE['nc'], in_maps, core_ids=list(range(B)),
                               **kw)
    _CACHE['exec_time_ns'] = res.exec_time_ns
    _CACHE['results'] = res
    out = np.stack([r['out_b'] for r in res.results], axis=1)
    return np.ascontiguousarray(out.astype(np.float32))
